# revision 1
# baseline (speedup 1.0000x reference)
"""Epipolar attention kernel for Trainium2 (8 NeuronCores, batch-parallel).

Host does O(B*3^3) geometry + O(N)/O(N*C) input prep (coefficient splits,
fp8 quantization of f_src, colsum, Lipschitz row-max bound); the device does
all O(N^2) / O(N^2*C) work:

  stage1 (i-rows on partitions):
      d[i,j]  = A_j x_i + B_j y_i + C_j        (PE, exact triple-bf16 split)
      xa      = |d|                            (DVE bitcast sign-mask)
      rhat_i  = sum_j exp(xa - Mhat_i)         (ACT, per-partition bias + accum)
  beta_i = Mhat_i + ln rhat_i                  (DVE exponent-field ln + tiny ops)
      broadcast beta across partitions         (PE transpose + one-hot matmuls)
  stage2 (j-rows on partitions):
      xT      = |dT| - beta_i                  (DVE fused scalar_tensor_tensor)
      u       = exp(xT)                        (ACT)          == e/r, exact softmax
      E2T     = exp(-u), S_j = sum_i E2T       (ACT + accum)
      W8      = E2T*(2^14/S_j) - 16            (DVE -> fp8e4)  [attn = 1/N + W/2^14]
  stage3 (c-blocks on partitions, outT = fs^T @ W^T):
      psum[c,i] = sum_j fs8[j,c] * W8[j,i]     (PE, fp8 DoubleRow, K=256/mm)
      outT = f16(psum + 2^14*F_c/N)            (ACT copy + per-partition bias)
Host: out = outT.T * 2^-14. The double softmax identity:
  softmax_i(1 - softmax_j(5(d-thre))) == E2/colsum(E2).
"""

import numpy as np
import ml_dtypes

import concourse.bass as bass
import concourse.bacc as bacc
import concourse.tile as tile
from concourse import mybir
from concourse.bass_utils import run_bass_kernel_spmd

B, C, H, W = 8, 1152, 32, 32
N = H * W           # 1024
P = 128
NT = N // P         # 8
CB = C // P         # 9
F32 = mybir.dt.float32
F16 = mybir.dt.float16
BF16 = mybir.dt.bfloat16
FP8 = mybir.dt.float8e4
I32 = mybir.dt.int32
BFNP = ml_dtypes.bfloat16
F8NP = ml_dtypes.float8_e4m3

SCALE = 16384.0     # 2^14 scaling of the centered attention W
LN2 = 0.6931471805599453

# log2(m) ~ c3*(m-r1)*(m^2 + pm*m + q) on [1,2]  (factored minimax cubic)
# fit: np.polyfit of log2 on [1,2] chebyshev nodes, deg 3, then factor.
_LOG2_COEF = np.polyfit(np.cos(np.pi * (np.arange(64) + .5) / 64) * .5 + 1.5,
                        np.log2(np.cos(np.pi * (np.arange(64) + .5) / 64) * .5 + 1.5), 3)
_C3, _C2, _C1, _C0 = [float(v) for v in _LOG2_COEF]
_ROOTS = np.roots(_LOG2_COEF)
_R1 = float(_ROOTS[np.argmin(np.abs(_ROOTS.imag))].real)     # real root (~1.0)
_QUAD = np.real(np.poly([r for r in _ROOTS
                         if abs(r - _R1) > 1e-9]))            # m^2 + pm*m + q
_PM, _Q = float(_QUAD[1]), float(_QUAD[2])

TRACE = False
LAST_RESULTS = None


# ----------------------------------------------------------------- device ---

def _build_nc():
    nc = bacc.Bacc()
    abc9 = nc.dram_tensor("abc9", (9, N), BF16, kind="ExternalInput")
    xy9 = nc.dram_tensor("xy9", (9, N), BF16, kind="ExternalInput")
    smallc = nc.dram_tensor("smallc", (P, NT + NT + CB), F32,
                            kind="ExternalInput")  # [-Mhat | Mhat-127*ln2 | F*2^14/N]
    oneh = nc.dram_tensor("oneh", (2 * NT, N), BF16, kind="ExternalInput")
    identD = nc.dram_tensor("ident", (P, P), BF16, kind="ExternalInput")
    fs8 = nc.dram_tensor("fs8", (N, C), FP8, kind="ExternalInput")
    outT = nc.dram_tensor("outT", (C, N), F16, kind="ExternalOutput")

    AF = mybir.ActivationFunctionType
    AO = mybir.AluOpType

    with tile.TileContext(nc) as tc:
        with (
            tc.tile_pool(name="consts", bufs=1) as consts,
            tc.tile_pool(name="persist", bufs=1) as persist,
            tc.tile_pool(name="pxa", bufs=4) as pxa,
            tc.tile_pool(name="pxT", bufs=3) as pxT,
            tc.tile_pool(name="pscr", bufs=1) as pscr,
            tc.tile_pool(name="pu", bufs=3) as pu,
            tc.tile_pool(name="pe2", bufs=4) as pe2,
            tc.tile_pool(name="posb", bufs=4) as posb,
            tc.tile_pool(name="stats", bufs=8) as stats,
            tc.tile_pool(name="P1", bufs=2, space="PSUM") as P1,
            tc.tile_pool(name="psC", bufs=4, space="PSUM") as psC,
        ):
            # ---- consts / inputs ----
            xy_sb = consts.tile([9, N], BF16, tag="xy")
            nc.sync.dma_start(out=xy_sb, in_=xy9[:, :])
            abc_sb = consts.tile([9, N], BF16, tag="abc")
            nc.scalar.dma_start(out=abc_sb[:, :512], in_=abc9[:, :512])
            nc.scalar.dma_start(out=abc_sb[:, 512:], in_=abc9[:, 512:])
            small_sb = consts.tile([P, NT + NT + CB], F32, tag="smallc")
            nc.sync.dma_start(out=small_sb, in_=smallc[:, :])
            mneg_sb = small_sb[:, 0:NT]
            mofs_sb = small_sb[:, NT:2 * NT]
            f14_sb = small_sb[:, 2 * NT:2 * NT + CB]
            oneh_sb = consts.tile([2 * NT, N], BF16, tag="oneh")
            nc.scalar.dma_start(out=oneh_sb, in_=oneh[:, :])
            ident = consts.tile([P, P], BF16, tag="ident")
            nc.scalar.dma_start(out=ident, in_=identD[:, :])

            fs8_sb = persist.tile([P, NT, C], FP8, tag="fs8")
            for s in range(NT):
                nc.sync.dma_start(out=fs8_sb[:, s, :], in_=fs8[s * P:(s + 1) * P, :])

            W8_sb = persist.tile([P, NT, N], FP8, tag="W8")
            dabs_all = persist.tile([P, NT, N], F32, tag="dabs")
            Bm_sb = persist.tile([P, N], F32, tag="Bm")
            bT_sb = persist.tile([2 * NT, P], BF16, tag="bT")
            bcol3 = persist.tile([P, 2 * NT], BF16, tag="bcol3")
            res1 = persist.tile([P, NT], F32, tag="res1")
            res2 = persist.tile([P, NT], F32, tag="res2")
            rcol = persist.tile([P, NT], F32, tag="rcol")
            bcol = persist.tile([P, NT], F32, tag="bcol")

            # preload the Exp ACT table before inputs land
            dummy = stats.tile([1, 2], F32, tag="dummy")
            nc.gpsimd.memset(dummy, 0.0)
            dummy2 = stats.tile([1, 2], F32, tag="dummy2")
            nc.scalar.activation(out=dummy2, in_=dummy, func=AF.Exp)

            # ---- stage 1: i-oriented pass -> rhat ----
            for it in range(NT):
                psd = P1.tile([P, N], F32, tag="psd")
                for h in range(2):
                    nc.tensor.matmul(
                        psd[:, h * 512:(h + 1) * 512],
                        lhsT=xy_sb[:, it * P:(it + 1) * P],
                        rhs=abc_sb[:, h * 512:(h + 1) * 512],
                        start=True, stop=True,
                    )
                xa = pxa.tile([P, N], F32, tag="xa")
                nc.vector.tensor_scalar(
                    out=xa.bitcast(I32), in0=psd.bitcast(I32),
                    scalar1=0x7FFFFFFF, scalar2=None, op0=AO.bitwise_and,
                )
                e_scr = pscr.tile([P, N], BF16, tag="escr")
                nc.scalar.activation(
                    out=e_scr, in_=xa, func=AF.Exp, bias=mneg_sb[:, it:it + 1],
                    accum_out=rcol[:, it:it + 1],
                )

            # ---- stage 2a (part 1): dT + |dT| for first stripes ----
            for u in range(0, 2):
                psd = P1.tile([P, N], F32, tag="psd")
                for h in range(2):
                    nc.tensor.matmul(
                        psd[:, h * 512:(h + 1) * 512],
                        lhsT=abc_sb[:, u * P:(u + 1) * P],
                        rhs=xy_sb[:, h * 512:(h + 1) * 512],
                        start=True, stop=True,
                    )
                nc.vector.tensor_scalar(
                    out=dabs_all[:, u, :].bitcast(I32), in0=psd.bitcast(I32),
                    scalar1=0x7FFFFFFF, scalar2=None, op0=AO.bitwise_and,
                )
            # ---- beta = Mhat + ln(rhat), via exponent-field log2 ----
            rI = rcol.bitcast(I32)
            expI = stats.tile([P, NT], I32, tag="expI")
            nc.vector.tensor_scalar(
                out=expI, in0=rI, scalar1=23, scalar2=None,
                op0=AO.logical_shift_right,
            )
            Ef = stats.tile([P, NT], F32, tag="Ef")
            nc.vector.tensor_copy(Ef, expI)     # int -> float convert
            mant = stats.tile([P, NT], F32, tag="mant")
            nc.vector.tensor_scalar(
                out=mant.bitcast(I32), in0=rI, scalar1=0x007FFFFF,
                scalar2=0x3F800000, op0=AO.bitwise_and, op1=AO.bitwise_or,
            )
            t1 = stats.tile([P, NT], F32, tag="t1")      # m - r1
            nc.vector.tensor_scalar(out=t1, in0=mant, scalar1=_R1, scalar2=None,
                                    op0=AO.subtract)
            t2 = stats.tile([P, NT], F32, tag="t2")      # m^2 + pm*m + q
            nc.vector.scalar_tensor_tensor(
                out=t2, in0=mant, scalar=_PM, in1=mant, op0=AO.add, op1=AO.mult)
            nc.vector.tensor_scalar(out=t2, in0=t2, scalar1=_Q, scalar2=None,
                                    op0=AO.add)
            t3 = stats.tile([P, NT], F32, tag="t3")      # c3*(m-r1)*(...) = log2(mant)
            nc.vector.tensor_tensor(out=t3, in0=t1, in1=t2, op=AO.mult)
            # t4 = log2(rhat) - 127 + ... -> Ef + c3*t3  (Ef holds exponent+127)
            t4 = stats.tile([P, NT], F32, tag="t4")
            nc.vector.scalar_tensor_tensor(
                out=t4, in0=t3, scalar=_C3, in1=Ef, op0=AO.mult, op1=AO.add)
            # bcol = t4*ln2 + (Mhat - 127*ln2)
            nc.vector.scalar_tensor_tensor(
                out=bcol, in0=t4, scalar=LN2, in1=mofs_sb, op0=AO.mult, op1=AO.add)

            # ---- broadcast beta to [128, N]: exact triple-bf16 split,
            # transpose, then one-hot bf16 matmuls (sum the 3 splits) ----
            nc.vector.tensor_copy(bcol3[:, 0:NT], bcol)
            nc.vector.tensor_tensor(out=res1, in0=bcol, in1=bcol3[:, 0:NT],
                                    op=AO.subtract)
            nc.vector.tensor_copy(bcol3[:, NT:2 * NT], res1)
            psT = P1.tile([2 * NT, P], BF16, tag="psd")
            nc.tensor.transpose(psT, bcol3, ident)
            nc.vector.tensor_copy(bT_sb, psT)
            psB = P1.tile([P, N], F32, tag="psd")
            for it in range(NT):
                nc.tensor.matmul(
                    psB[:, it * P:(it + 1) * P],
                    lhsT=oneh_sb[:, it * P:(it + 1) * P],
                    rhs=bT_sb,
                    start=True, stop=True,
                )
            nc.scalar.activation(out=Bm_sb, in_=psB, func=AF.Copy)
            # first two beta-subtracts, half on DVE half on Pool, so the
            # stage-2 ACT chain starts as early as possible
            xt01 = []
            for u in range(2):
                xTe = pxT.tile([P, N], BF16, tag="xT", name=f"xTe{u}")
                nc.vector.tensor_tensor(
                    out=xTe[:, :512], in0=dabs_all[:, u, :512],
                    in1=Bm_sb[:, :512], op=AO.subtract)
                nc.gpsimd.tensor_tensor(
                    out=xTe[:, 512:], in0=dabs_all[:, u, 512:],
                    in1=Bm_sb[:, 512:], op=AO.subtract)
                xt01.append(xTe)

            # ---- stage 2b: j-oriented softmax chain -> W8 ----
            # Hand-scheduled emission: DVE takes subs 0,1,6,7 + W8 bundles 0-3;
            # Pool takes subs 2-5 + W8 bundles 4-7 (keeps ACT exp pipeline fed
            # and gets W8(0) out early so the GEMM stream starts sooner).
            xts = [None] * NT
            uts = [None] * NT
            e2s = [None] * NT
            S1s = [None] * NT
            invs = [None] * NT

            def emit_sub(u, eng):
                xts[u] = pxT.tile([P, N], BF16, tag="xT", name=f"xT{u}")
                eng.tensor_tensor(out=xts[u], in0=dabs_all[:, u, :], in1=Bm_sb,
                                  op=AO.subtract)

            def emit_exps(u):
                uts[u] = pu.tile([P, N], BF16, tag="uT", name=f"uT{u}")
                nc.scalar.activation(out=uts[u], in_=xts[u], func=AF.Exp)
                S1s[u] = stats.tile([P, 1], F32, tag="S1", name=f"S1{u}")
                e2s[u] = pe2.tile([P, N], F16, tag="E2T", name=f"E2T{u}")
                nc.scalar.activation(
                    out=e2s[u], in_=uts[u], func=AF.Exp, scale=-1.0,
                    accum_out=S1s[u],
                )

            def emit_recip(u):
                invs[u] = stats.tile([P, 1], F32, tag="invS", name=f"invS{u}")
                nc.vector.reciprocal(invs[u], S1s[u])

            def emit_w8(u, eng):
                invS14 = stats.tile([P, 1], F32, tag="invS14", name=f"iS14{u}")
                eng.tensor_scalar_mul(invS14, invs[u], SCALE)
                eng.tensor_scalar(
                    out=W8_sb[:, u, :], in0=e2s[u], scalar1=invS14,
                    scalar2=SCALE / N, op0=AO.mult, op1=AO.subtract,
                )

            # ---- stage 2a (part 2): remaining dT stripes ----
            for u in range(2, NT):
                psd = P1.tile([P, N], F32, tag="psd", name=f"psd2_{u}")
                for h in range(2):
                    nc.tensor.matmul(
                        psd[:, h * 512:(h + 1) * 512],
                        lhsT=abc_sb[:, u * P:(u + 1) * P],
                        rhs=xy_sb[:, h * 512:(h + 1) * 512],
                        start=True, stop=True,
                    )
                nc.vector.tensor_scalar(
                    out=dabs_all[:, u, :].bitcast(I32), in0=psd.bitcast(I32),
                    scalar1=0x7FFFFFFF, scalar2=None, op0=AO.bitwise_and,
                )

            # ---- stage 2b: j-oriented softmax chain -> W8 ----
            for u in range(NT):
                if u < 2:
                    xT = xt01[u]
                else:
                    xT = pxT.tile([P, N], BF16, tag="xT", name=f"xT{u}")
                    sub_eng = nc.gpsimd if u < 6 else nc.vector
                    sub_eng.tensor_tensor(out=xT, in0=dabs_all[:, u, :],
                                          in1=Bm_sb, op=AO.subtract)
                uT = pu.tile([P, N], BF16, tag="uT", name=f"uT{u}")
                nc.scalar.activation(out=uT, in_=xT, func=AF.Exp)
                S1 = stats.tile([P, 1], F32, tag="S1", name=f"S1{u}")
                E2T = pe2.tile([P, N], F16, tag="E2T", name=f"E2T{u}")
                nc.scalar.activation(
                    out=E2T, in_=uT, func=AF.Exp, scale=-1.0, accum_out=S1,
                )
                invS = stats.tile([P, 1], F32, tag="invS", name=f"invS{u}")
                nc.vector.reciprocal(invS, S1)
                invS14 = stats.tile([P, 1], F32, tag="invS14", name=f"iS14{u}")
                nc.vector.tensor_scalar_mul(invS14, invS, SCALE)
                nc.vector.tensor_scalar(
                    out=W8_sb[:, u, :], in0=E2T, scalar1=invS14,
                    scalar2=SCALE / N, op0=AO.mult, op1=AO.subtract,
                )

            # ---- stage 3: outT[c,i] = sum_j fs8[j,c]*W8[j,i] (+ F term) ----
            for cb in range(CB):
                for ic in range(2):
                    ps = psC.tile([P, 512], F32, tag="oc")
                    for s in range(4):
                        nc.tensor.matmul(
                            ps,
                            lhsT=fs8_sb[:, 2 * s:2 * s + 2, cb * P:(cb + 1) * P],
                            rhs=W8_sb[:, 2 * s:2 * s + 2, ic * 512:(ic + 1) * 512],
                            start=(s == 0), stop=(s == 3),
                            perf_mode=mybir.MatmulPerfMode.DoubleRow,
                        )
                    osb = posb.tile([P, 512], F16, tag="osb")
                    nc.scalar.activation(
                        out=osb, in_=ps, func=AF.Identity,
                        bias=f14_sb[:, cb:cb + 1],
                    )
                    nc.sync.dma_start(
                        out=outT[cb * P:(cb + 1) * P, ic * 512:(ic + 1) * 512],
                        in_=osb,
                    )
    nc.compile()
    return nc


_NC = None


def _get_nc():
    global _NC
    if _NC is None:
        _NC = _build_nc()
    return _NC


# ------------------------------------------------------------------- host ---

def _skew(t):
    z = np.zeros_like(t[:, 0])
    return np.stack([
        np.stack([z, -t[:, 2], t[:, 1]], -1),
        np.stack([t[:, 2], z, -t[:, 0]], -1),
        np.stack([-t[:, 1], t[:, 0], z], -1),
    ], 1)


def _fundamental(K1, K2, R, t):
    E = _skew(t) @ R
    U, S, Vt = np.linalg.svd(E)
    S = S.copy()
    S[:, 2] = 0.0
    E = U @ (S[:, :, None] * Vt)
    return np.linalg.inv(np.swapaxes(K2, 1, 2)) @ E @ np.linalg.inv(K1)


def _split3(v):
    """Triple bf16 split: v ~= hi + mid + lo (24 mantissa bits)."""
    v = v.astype(np.float32)
    hi = v.astype(BFNP)
    r1 = v - hi.astype(np.float32)
    mid = r1.astype(BFNP)
    r2 = r1 - mid.astype(np.float32)
    lo = r2.astype(BFNP)
    return hi, mid, lo


def _host_prep(f_src, K1, K2, R, t):
    ix, iy = np.meshgrid(np.arange(H, dtype=np.float32),
                         np.arange(W, dtype=np.float32), indexing="ij")
    x = ix.ravel()
    y = iy.ravel()
    comb = np.stack([x, y, np.ones(N, np.float32)], 0)  # (3,N)

    F = _fundamental(K1, K2, R, t)                    # (B,3,3)
    lines = (F @ comb).astype(np.float32)             # (B,3,N)
    lines = lines / lines[:, 2:3, :]
    y0 = -lines[:, 2, :] / lines[:, 1, :]
    y1 = -(lines[:, 2, :] + lines[:, 0, :] * np.float32(W)) / lines[:, 1, :]
    dy = y0 - y1
    L = np.sqrt(np.float32(W * W) + dy * dy)
    A5 = np.float32(5.0) * (dy / L)
    B5 = np.float32(5.0) * (np.float32(W) / L)
    C5 = np.float32(-5.0) * (np.float32(W) * y0 / L)

    Ah, Am, Al = _split3(A5)
    Bh, Bm, Bl = _split3(B5)
    Ch, Cm, Cl = _split3(C5)
    abc9 = np.stack([Ah, Bh, Ch, Am, Bm, Cm, Al, Bl, Cl], axis=1)  # (B,9,N)
    xy9 = np.tile(comb, (3, 1)).astype(BFNP)                        # (9,N)

    # Lipschitz bound on the row max: |grad d5| = 5 exactly, so
    # Mhat_i = max_j d5(coarse pt) + 5*dist is within [m_i, m_i + 56.6].
    gx = np.array([4., 12., 20., 28.], np.float32)
    cgx, cgy = np.meshgrid(gx, gx, indexing="ij")
    cgx = cgx.ravel()[:, None]
    cgy = cgy.ravel()[:, None]                                       # (16,1)
    dc = np.abs(A5[:, None, :] * cgx[None] + B5[:, None, :] * cgy[None]
                + C5[:, None, :])                                    # (B,16,N)
    mc = dc.max(-1)                                                  # (B,16)
    d2 = (x[None, :] - cgx) ** 2 + (y[None, :] - cgy) ** 2           # (16,N)
    near = np.argmin(d2, axis=0)                                     # (N,)
    dist = np.sqrt(d2[near, np.arange(N)])
    Mhat = mc[:, near] + np.float32(5.0) * dist[None, :]             # (B,N)

    mneg = -Mhat.reshape(B, NT, P).transpose(0, 2, 1)                # (B,128,8)
    mofs = (Mhat - np.float32(127.0 * LN2)).reshape(B, NT, P).transpose(0, 2, 1)

    fs = f_src.reshape(B, C, N).transpose(0, 2, 1)                   # (B,N,C)
    fs8 = np.clip(fs, -240, 240).astype(F8NP)
    Fcol = fs.astype(np.float64).sum(axis=1) * (SCALE / N)           # (B,C)
    f14 = Fcol.astype(np.float32).reshape(B, CB, P).transpose(0, 2, 1)  # (B,128,9)
    smallc = np.concatenate([mneg, mofs, f14], axis=2).astype(np.float32)
    return abc9, xy9, smallc, fs8


_ONEH = None
_IDENT = None


def _consts():
    global _ONEH, _IDENT
    if _ONEH is None:
        oneh = np.zeros((2 * NT, N), BFNP)
        for s in range(2):
            for k in range(NT):
                oneh[s * NT + k, k * P:(k + 1) * P] = 1.0
        _ONEH = oneh
        _IDENT = np.eye(P, dtype=BFNP)
    return _ONEH, _IDENT


def host_prep_all(f_src, K1, K2, R, t):
    abc9, xy9, smallc, fs8 = _host_prep(f_src, K1, K2, R, t)
    oneh, ident = _consts()
    in_maps = [
        {"abc9": np.ascontiguousarray(abc9[b]), "xy9": xy9,
         "smallc": np.ascontiguousarray(smallc[b]),
         "oneh": oneh, "ident": ident,
         "fs8": np.ascontiguousarray(fs8[b])}
        for b in range(B)
    ]
    return in_maps


def finish(outT_list):
    outs = np.stack([o.astype(np.float32).T for o in outT_list], 0)  # (B,N,C)
    outs *= np.float32(1.0 / SCALE)
    return outs.reshape(B, C, H, W)


def kernel(f_tar=None, f_src=None, K1=None, K2=None, R=None, t=None):
    global LAST_RESULTS
    f_src = np.asarray(f_src, np.float32)
    K1 = np.asarray(K1, np.float32)
    K2 = np.asarray(K2, np.float32)
    R = np.asarray(R, np.float32)
    t = np.asarray(t, np.float32)

    in_maps = host_prep_all(f_src, K1, K2, R, t)
    res = run_bass_kernel_spmd(_get_nc(), in_maps, list(range(B)), trace=TRACE)
    LAST_RESULTS = res
    return finish([res.results[b]["outT"] for b in range(B)])

